# revision 1
# baseline (speedup 1.0000x reference)
"""Self-contained Trainium2 Bass kernel for nn_CrossStageAttention.

Data-parallel over batch: 16 images -> 8 NeuronCores x 2 images each.
Training-mode BatchNorm statistics are made global via two tiny AllReduces.

All heavy matmuls run as float32r on the PE array. The torch
"(attn@v).transpose(1,2).reshape" scramble is absorbed into the fuse access
patterns (o_nat orientation): catT[i, pos=2u+v] = o_nat[512v+i, u].
"""
import numpy as np
from contextlib import ExitStack

import concourse.bass as bass
import concourse.tile as tile
import concourse.bacc as bacc
from concourse import mybir, masks
from concourse.bass_utils import run_bass_kernel_spmd

N_CORES = 8
IMGS = 2
C = 512
N = 1024          # query positions per image (32x32)
PC = 256
MP = 4096         # prev positions per image (64x64)
F32 = mybir.dt.float32
F32R = mybir.dt.float32r
SCALE = 32 ** -0.5
B0_SELF = 128.0   # constant softmax-stabilization bias for self-attention
EPS = 1e-5
INV_CNT = 1.0 / (16 * 1024)
AF = mybir.ActivationFunctionType
ALU = mybir.AluOpType
X_AXIS = mybir.AxisListType.X


def build_nc():
    nc = bacc.Bacc("TRN2", target_bir_lowering=False, debug=False,
                   num_devices=N_CORES)
    x_d = nc.dram_tensor("x", [IMGS, N, C], F32R, kind="ExternalInput").ap()
    px_d = nc.dram_tensor("px", [IMGS, MP, PC], F32R, kind="ExternalInput").ap()
    wq_d = nc.dram_tensor("wq", [C, C], F32R, kind="ExternalInput").ap()
    wp_d = nc.dram_tensor("wp", [PC, C], F32R, kind="ExternalInput").ap()
    fw_d = nc.dram_tensor("fw", [2 * C, C], F32R, kind="ExternalInput").ap()
    ow_d = nc.dram_tensor("ow", [9, C, C], F32R, kind="ExternalInput").ap()
    g1_d = nc.dram_tensor("g1", [128, 4], F32, kind="ExternalInput").ap()
    b1_d = nc.dram_tensor("b1", [128, 4], F32, kind="ExternalInput").ap()
    g2_d = nc.dram_tensor("g2", [1, C], F32, kind="ExternalInput").ap()
    b2_d = nc.dram_tensor("b2", [1, C], F32, kind="ExternalInput").ap()
    pars_d = nc.dram_tensor("pars", [1, 2], F32, kind="ExternalInput").ap()
    out_d = nc.dram_tensor("out", [IMGS, N, C], F32, kind="ExternalOutput").ap()

    with tile.TileContext(nc) as tc, ExitStack() as ctx:
        const = ctx.enter_context(tc.tile_pool(name="const", bufs=1))
        scr = ctx.enter_context(tc.tile_pool(name="scr", bufs=10))   # [128,512] scratch
        ld = ctx.enter_context(tc.tile_pool(name="ld", bufs=3))
        sm = ctx.enter_context(tc.tile_pool(name="sm", bufs=10))
        ps = ctx.enter_context(tc.tile_pool(name="ps", bufs=8, space="PSUM"))
        dram = ctx.enter_context(tc.tile_pool(name="dram", bufs=1, space="DRAM"))

        # ------------- DRAM scratch -------------
        xT_d = dram.tile([IMGS, 4, 128, N], F32R, tag="xT_d")
        fusx_d = dram.tile([IMGS, 4, 2, 128, 512], F32, tag="fusx_d")
        y_d = dram.tile([IMGS, 8, 128, C], F32R, tag="y_d")
        bn1_in = dram.tile([128, 8], F32, tag="bn1i")
        bn1_out = dram.tile([128, 8], F32, tag="bn1o")
        bn2_in = dram.tile([1, 1024], F32, tag="bn2i")
        bn2_out = dram.tile([1, 1024], F32, tag="bn2o")

        # ------------- constants / params -------------
        identF = const.tile([128, 128], F32, tag="identF")
        masks.make_identity(nc, identF[:])
        ident = const.tile([128, 128], F32R, tag="ident")
        nc.vector.tensor_copy(ident[:], identF[:])
        onesF = const.tile([128, 2], F32, tag="onesF")
        nc.gpsimd.memset(onesF[:], 1.0)
        ones_r = const.tile([128, 1], F32R, tag="ones")
        nc.vector.tensor_copy(ones_r[:], onesF[:, 0:1])
        ones2 = const.tile([128, 2], F32R, tag="ones2")
        nc.vector.tensor_copy(ones2[:], onesF[:])
        zrow = const.tile([128, 34], F32, tag="zrow")
        nc.gpsimd.memset(zrow[:], 0.0)
        b0s = const.tile([128, 1], F32, tag="b0s")
        nc.gpsimd.memset(b0s[:], -B0_SELF)
        eps_t = const.tile([128, 1], F32, tag="eps")
        nc.gpsimd.memset(eps_t[:], EPS)
        g1_s = const.tile([128, 4], F32, tag="g1")
        b1_s = const.tile([128, 4], F32, tag="b1")
        pars_s = const.tile([1, 2], F32, tag="pars")
        pars_bc = const.tile([128, 2], F32, tag="parsbc")
        s1acc = const.tile([128, 4, 4], F32, tag="s1acc")
        ss1acc = const.tile([128, 4, 4], F32, tag="ss1acc")
        s1v = const.tile([128, 4], F32, tag="s1v")
        t1v = const.tile([128, 4], F32, tag="t1v")
        nc.sync.dma_start(g1_s[:], g1_d)
        nc.sync.dma_start(b1_s[:], b1_d)
        nc.sync.dma_start(pars_s[:], pars_d)
        nc.gpsimd.partition_broadcast(pars_bc[:], pars_s[:])

        def transpose_to(dst_ap, src_ap, eng):
            pt = ps.tile([128, 512], F32R, tag="b", name="tp")
            nc.tensor.transpose(pt[:, 0:128], src_ap, ident[:])
            if eng == "act":
                nc.scalar.copy(dst_ap, pt[:, 0:128])
            else:
                nc.vector.tensor_copy(dst_ap, pt[:, 0:128])

        # =================== attention scope ===================
        with tc.tile_pool(name="attn", bufs=1) as ap_:
            wq_s = ap_.tile([128, 4, C], F32R, tag="wq", name="wq")
            wp_s = ap_.tile([128, 2, C], F32R, tag="wp", name="wp")
            fw_s = ap_.tile([128, 8, C], F32R, tag="fw", name="fw")
            nc.sync.dma_start(wq_s[:], wq_d.rearrange("(ic p) c -> p ic c", p=128))
            nc.sync.dma_start(wp_s[:], wp_d.rearrange("(ic p) c -> p ic c", p=128))
            nc.sync.dma_start(fw_s[:], fw_d.rearrange("(ic p) o -> p ic o", p=128))
            qT_t = None
            for img in range(IMGS):
                qT_t = ap_.tile([128, 4, N], F32R, tag="qT", name="qT")
                xnow_t = ap_.tile([128, 8, C], F32R, tag="xnow", name="xnow")
                xprev_t = ap_.tile([128, 8, C], F32R, tag="xprev", name="xprev")

                def do_attn(kind, kvT, vnat):
                    bias = b0s[:] if kind == "self" else 0.0
                    for nh in range(2):
                        eas = []
                        for mi in range(8):
                            lg = ps.tile([128, 512], F32, tag="b", name="lg")
                            for ci in range(4):
                                nc.tensor.matmul(
                                    lg[:],
                                    kvT[:, ci, 128 * mi:128 * mi + 128],
                                    qT_t[:, ci, 512 * nh:512 * nh + 512],
                                    start=(ci == 0), stop=(ci == 3))
                            ea = scr.tile([128, 512], F32R, tag="s", name="ea")
                            nc.scalar.activation(ea[:], lg[:], AF.Exp,
                                                 bias=bias, scale=SCALE)
                            eas.append(ea)
                        for np2 in range(2):
                            o_ps = [ps.tile([128, 512], F32, tag="b", name="ops")
                                    for _ in range(2)]
                            s_ps = [ps.tile([128, 512], F32, tag="b", name="sps")
                                    for _ in range(2)]
                            for mi in range(8):
                                for k in range(2):
                                    lhsT = eas[mi][:, 128 * (2 * np2 + k):
                                                   128 * (2 * np2 + k) + 128]
                                    nc.tensor.matmul(o_ps[k][:], lhsT,
                                                     vnat[:, mi, :],
                                                     start=(mi == 0),
                                                     stop=(mi == 7))
                                    nc.tensor.matmul(s_ps[k][:, 0:2], lhsT,
                                                     ones2[:],
                                                     start=(mi == 0),
                                                     stop=(mi == 7))
                            for k in range(2):
                                nck = 4 * nh + 2 * np2 + k
                                rec = sm.tile([128, 4], F32, name="rec")
                                nc.vector.reciprocal(rec[:, 0:1],
                                                     s_ps[k][:, 0:1])
                                if kind == "self":
                                    nc.vector.tensor_scalar_mul(
                                        xnow_t[:, nck, :], o_ps[k][:],
                                        rec[:, 0:1])
                                elif kind == "avg":
                                    w = sm.tile([128, 4], F32, name="bw")
                                    nc.vector.tensor_tensor(
                                        w[:, 0:1], rec[:, 0:1],
                                        pars_bc[:, 0:1], op=ALU.mult)
                                    nc.vector.tensor_scalar_mul(
                                        xprev_t[:, nck, :], o_ps[k][:],
                                        w[:, 0:1])
                                else:
                                    w = sm.tile([128, 4], F32, name="bw")
                                    nc.vector.tensor_tensor(
                                        w[:, 0:1], rec[:, 0:1],
                                        pars_bc[:, 1:2], op=ALU.mult)
                                    t_ = scr.tile([128, 512], F32, tag="s", name="mx")
                                    nc.vector.tensor_scalar_mul(
                                        t_[:], o_ps[k][:], w[:, 0:1])
                                    nc.vector.tensor_tensor(
                                        xprev_t[:, nck, :],
                                        xprev_t[:, nck, :], t_[:], op=ALU.add)

                # ---- xT (PE transposes), spilled to DRAM; qT projection
                xT_t = ap_.tile([128, 4, N], F32R, tag="xTtr", name="xTtr")
                for nt in range(8):
                    xl = ld.tile([128, C], F32R, tag="xload", name="xl")
                    nc.sync.dma_start(xl[:], x_d[img, 128 * nt:128 * nt + 128, :])
                    for ic in range(4):
                        transpose_to(xT_t[:, ic, 128 * nt:128 * nt + 128],
                                     xl[:, 128 * ic:128 * ic + 128], "vec")
                for ic in range(4):
                    nc.sync.dma_start(xT_d[img, ic], xT_t[:, ic, :])
                for ci in range(4):
                    for nh in range(2):
                        qp = ps.tile([128, 512], F32, tag="b", name="qp")
                        for ic in range(4):
                            nc.tensor.matmul(
                                qp[:], wq_s[:, ic, 128 * ci:128 * ci + 128],
                                xT_t[:, ic, 512 * nh:512 * nh + 512],
                                start=(ic == 0), stop=(ic == 3))
                        nc.scalar.copy(qT_t[:, ci, 512 * nh:512 * nh + 512],
                                       qp[:])
                # ---- qkv natural (tag "nat" slot shared with avg/max nat)
                nat_t = ap_.tile([128, 8, C], F32R, tag="nat", name="qkvnat")
                for mi in range(8):
                    for ci in range(4):
                        transpose_to(nat_t[:, mi, 128 * ci:128 * ci + 128],
                                     qT_t[:, ci, 128 * mi:128 * mi + 128],
                                     "act")
                do_attn("self", qT_t, nat_t)

                # ---- prevqkvT chunks (chunked prevx transposes) + pooling
                avgT_t = ap_.tile([128, 4, N], F32R, tag="avgT", name="avgT")
                maxT_t = ap_.tile([128, 4, N], F32R, tag="maxT", name="maxT")
                for ch in range(8):
                    pxc = ap_.tile([128, 2, 512], F32R, tag="pxc", bufs=2,
                                   name="pxc")
                    for kk in range(4):
                        pl = ld.tile([128, PC], F32R, tag="pxload", name="pl")
                        nc.sync.dma_start(
                            pl[:],
                            px_d[img, 512 * ch + 128 * kk:
                                 512 * ch + 128 * kk + 128, :])
                        for pc in range(2):
                            transpose_to(pxc[:, pc, 128 * kk:128 * kk + 128],
                                         pl[:, 128 * pc:128 * pc + 128],
                                         "vec")
                    for ci in range(4):
                        pq = ps.tile([128, 512], F32, tag="b", name="pq")
                        for pc in range(2):
                            nc.tensor.matmul(
                                pq[:], wp_s[:, pc, 128 * ci:128 * ci + 128],
                                pxc[:, pc, :],
                                start=(pc == 0), stop=(pc == 1))
                        pqs = scr.tile([128, 512], F32, tag="s", name="pqs")
                        nc.scalar.copy(pqs[:], pq[:])
                        v = pqs[:].rearrange("p (i a j b) -> p i a j b",
                                             i=4, a=2, j=32, b=2)
                        pm = scr.tile([128, 512], F32, tag="s", name="pm")
                        pm1 = pm[:, 0:256].rearrange(
                            "p (i a j) -> p i a j", i=4, a=2)
                        nc.vector.tensor_tensor(pm1, v[:, :, :, :, 0],
                                                v[:, :, :, :, 1], op=ALU.max)
                        nc.vector.tensor_tensor(
                            maxT_t[:, ci, 128 * ch:128 * ch + 128]
                            .rearrange("p (i j) -> p i j", j=32),
                            pm1[:, :, 0, :], pm1[:, :, 1, :], op=ALU.max)
                        pa = scr.tile([128, 512], F32, tag="s", name="pa")
                        pa1 = pa[:, 0:256].rearrange(
                            "p (i a j) -> p i a j", i=4, a=2)
                        nc.vector.tensor_tensor(pa1, v[:, :, :, :, 0],
                                                v[:, :, :, :, 1], op=ALU.add)
                        pa2 = pa[:, 256:384].rearrange(
                            "p (i j) -> p i j", j=32)
                        nc.vector.tensor_tensor(pa2, pa1[:, :, 0, :],
                                                pa1[:, :, 1, :], op=ALU.add)
                        nc.scalar.mul(
                            avgT_t[:, ci, 128 * ch:128 * ch + 128]
                            .rearrange("p (i j) -> p i j", j=32),
                            pa2, 0.25)
                # ---- avg attention
                nat_t = ap_.tile([128, 8, C], F32R, tag="nat", name="avgnat")
                for mi in range(8):
                    for ci in range(4):
                        transpose_to(nat_t[:, mi, 128 * ci:128 * ci + 128],
                                     avgT_t[:, ci, 128 * mi:128 * mi + 128],
                                     "act")
                do_attn("avg", avgT_t, nat_t)
                # ---- max attention
                nat_t = ap_.tile([128, 8, C], F32R, tag="nat", name="maxnat")
                for mi in range(8):
                    for ci in range(4):
                        transpose_to(nat_t[:, mi, 128 * ci:128 * ci + 128],
                                     maxT_t[:, ci, 128 * mi:128 * mi + 128],
                                     "act")
                do_attn("max", maxT_t, nat_t)

                # ---- fuse matmul + BN1 partial stats, fusx spilled
                for oi in range(4):
                    for v in range(2):
                        fp = ps.tile([128, 512], F32, tag="b", name="fp")
                        for ii in range(8):
                            rhs = (xnow_t[:, 4 * v + ii, :] if ii < 4
                                   else xprev_t[:, 4 * v + (ii - 4), :])
                            nc.tensor.matmul(
                                fp[:], fw_s[:, ii, 128 * oi:128 * oi + 128],
                                rhs, start=(ii == 0), stop=(ii == 7))
                        slot = 2 * img + v
                        fsb = scr.tile([128, 512], F32, tag="s", name="fsb")
                        nc.scalar.copy(fsb[:], fp[:])
                        nc.sync.dma_start(fusx_d[img, oi, v], fsb[:])
                        nc.vector.tensor_reduce(
                            s1acc[:, oi, slot:slot + 1], fp[:],
                            axis=X_AXIS, op=ALU.add)
                        fsq = scr.tile([128, 512], F32, tag="s", name="fsq")
                        nc.scalar.square(fsq[:], fp[:])
                        nc.vector.tensor_reduce(
                            ss1acc[:, oi, slot:slot + 1], fsq[:],
                            axis=X_AXIS, op=ALU.add)

        # =================== BN1 global stats ===================
        sum1 = sm.tile([128, 4], F32, name="sum1")
        ssq1 = sm.tile([128, 4], F32, name="ssq1")
        nc.vector.tensor_reduce(sum1[:], s1acc[:], axis=X_AXIS, op=ALU.add)
        nc.vector.tensor_reduce(ssq1[:], ss1acc[:], axis=X_AXIS, op=ALU.add)
        nc.gpsimd.dma_start(bn1_in[:, 0:4], sum1[:])
        nc.gpsimd.dma_start(bn1_in[:, 4:8], ssq1[:])
        nc.gpsimd.collective_compute(
            "AllReduce", ALU.add, replica_groups=[list(range(N_CORES))],
            ins=[bn1_in.opt()], outs=[bn1_out.opt()])
        allst = sm.tile([128, 8], F32, name="allst")
        nc.sync.dma_start(allst[:], bn1_out[:])
        mean1 = sm.tile([128, 4], F32, name="mean1")
        tA = sm.tile([128, 4], F32, name="tA")
        tB = sm.tile([128, 4], F32, name="tB")
        nc.scalar.mul(mean1[:], allst[:, 0:4], INV_CNT)
        nc.scalar.mul(tA[:], allst[:, 4:8], INV_CNT)
        nc.scalar.square(tB[:], mean1[:])
        nc.vector.tensor_tensor(tA[:], tA[:], tB[:], op=ALU.subtract)
        nc.scalar.activation(tA[:], tA[:], AF.Sqrt, bias=eps_t[:])
        nc.vector.reciprocal(tA[:], tA[:])
        nc.vector.tensor_tensor(s1v[:], g1_s[:], tA[:], op=ALU.mult)
        nc.vector.tensor_tensor(tB[:], mean1[:], s1v[:], op=ALU.mult)
        nc.vector.tensor_tensor(t1v[:], b1_s[:], tB[:], op=ALU.subtract)

        # =================== conv scope ===================
        with tc.tile_pool(name="conv", bufs=1) as cp_, \
                tc.tile_pool(name="rows", bufs=8) as rows:
            ow_s = cp_.tile([128, 9, 4, C], F32R, tag="ow", name="ow")
            nc.sync.dma_start(ow_s[:],
                              ow_d.rearrange("t (ic p) o -> p t ic o", p=128))
            # ---- BN1 apply + residual -> 3 shifted vert-padded x2 buffers
            # x2s[d][(h+1)*32 + w - d + 1] = x2[h, w]   (d = kernel dw)
            x2shs = []
            for img in range(IMGS):
                x2s = cp_.tile([128, 3, 4, 1088], F32R, tag="x2s",
                               name=f"x2s{img}")
                x2shs.append(x2s)
                for d in range(3):
                    for ci in range(4):
                        x2v = (x2s[:, d, ci]
                               .rearrange("p (ph pw) -> p ph pw", pw=32))
                        nc.vector.tensor_copy(x2v[:, 0, :], zrow[:, 0:32])
                        nc.vector.tensor_copy(x2v[:, 33, :], zrow[:, 0:32])
                        if d == 0:
                            nc.vector.tensor_copy(x2v[:, 1:33, 0],
                                                  zrow[:, 0:32])
                        if d == 2:
                            nc.vector.tensor_copy(x2v[:, 1:33, 31],
                                                  zrow[:, 0:32])
                for oi in range(4):
                    xr = cp_.tile([128, N], F32R, tag="xrld", bufs=2, name="xr")
                    nc.sync.dma_start(xr[:], xT_d[img, oi])
                    for v in range(2):
                        fr = scr.tile([128, 512], F32, tag="s", name="fr")
                        nc.sync.dma_start(fr[:], fusx_d[img, oi, v])
                        rt = scr.tile([128, 512], F32, tag="s", name="rt")
                        nc.scalar.activation(rt[:], fr[:], AF.Relu,
                                             bias=t1v[:, oi:oi + 1],
                                             scale=s1v[:, oi:oi + 1])
                        rtv = rt[:].rearrange("p (a b) -> p a b", b=16)
                        xin = (xr[:].rearrange("p (a b two) -> p a b two",
                                               a=32, two=2)[:, :, :, v])
                        for d in range(3):
                            d0 = v - d + 1
                            pad = (x2s[:, d, oi]
                                   .rearrange("p (ph pw) -> p ph pw", pw=32)
                                   [:, 1:33, :]
                                   .rearrange("p a (b2 two) -> p a b2 two",
                                              two=2))
                            if d0 == 2:
                                dst, sl = pad[:, :, 1:16, 0], slice(0, 15)
                            elif d0 == 1:
                                dst, sl = pad[:, :, 0:16, 1], slice(0, 16)
                            elif d0 == 0:
                                dst, sl = pad[:, :, 0:16, 0], slice(0, 16)
                            else:  # d0 == -1
                                dst, sl = pad[:, :, 0:15, 1], slice(1, 16)
                            nc.vector.tensor_tensor(
                                dst, rtv[:, :, sl], xin[:, :, sl], op=ALU.add)

            # ---- conv 3x3 + BN2 stats (y spilled)
            sy_ps = ps.tile([128, 512], F32, tag="b", name="syp")
            sy2_ps = ps.tile([128, 512], F32, tag="b", name="sy2p")
            first = True
            for img in range(IMGS):
                for t in range(8):
                    yp = ps.tile([128, 512], F32, tag="b", name="yp")
                    k = 0
                    for tap in range(9):
                        dh, dw = tap // 3, tap % 3
                        for ii in range(4):
                            lhsT = x2shs[img][:, dw, ii,
                                              32 * (4 * t + dh):
                                              32 * (4 * t + dh) + 128]
                            nc.tensor.matmul(yp[:], lhsT, ow_s[:, tap, ii, :],
                                             start=(k == 0), stop=(k == 35))
                            k += 1
                    ysb = scr.tile([128, 512], F32R, tag="s", name="ysb")
                    nc.scalar.copy(ysb[:], yp[:])
                    nc.sync.dma_start(y_d[img, t], ysb[:])
                    ysq = scr.tile([128, 512], F32R, tag="s", name="ysq")
                    nc.scalar.square(ysq[:], yp[:])
                    last = (img == IMGS - 1 and t == 7)
                    nc.tensor.matmul(sy_ps[0:1, :], ones_r[:], ysb[:],
                                     start=first, stop=last)
                    nc.tensor.matmul(sy2_ps[0:1, :], ones_r[:], ysq[:],
                                     start=first, stop=last)
                    first = False

            # ---- BN2 global stats
            syr = rows.tile([1, 512], F32, tag="r2k", name="syr")
            sy2r = rows.tile([1, 512], F32, tag="r2k", name="sy2r")
            nc.vector.tensor_copy(syr[:], sy_ps[0:1, :])
            nc.vector.tensor_copy(sy2r[:], sy2_ps[0:1, :])
            nc.gpsimd.dma_start(bn2_in[:, 0:512], syr[:])
            nc.gpsimd.dma_start(bn2_in[:, 512:1024], sy2r[:])
            nc.gpsimd.collective_compute(
                "AllReduce", ALU.add, replica_groups=[list(range(N_CORES))],
                ins=[bn2_in.opt()], outs=[bn2_out.opt()])
            st2 = rows.tile([1, 1024], F32, tag="st2", bufs=1, name="st2")
            nc.sync.dma_start(st2[:], bn2_out[:])
            g2_s = rows.tile([1, C], F32, tag="r2k", name="g2s")
            b2_s = rows.tile([1, C], F32, tag="r2k", name="b2s")
            nc.sync.dma_start(g2_s[:], g2_d)
            nc.sync.dma_start(b2_s[:], b2_d)
            mean2 = rows.tile([1, 512], F32, tag="r2k", name="mean2")
            s2v = rows.tile([1, 512], F32, tag="r2k", name="s2v")
            t2v = rows.tile([1, 512], F32, tag="r2k", name="t2v")
            u1 = rows.tile([1, 512], F32, tag="r2k", name="u1")
            u2 = rows.tile([1, 512], F32, tag="r2k", name="u2")
            nc.scalar.mul(mean2[:], st2[:, 0:512], INV_CNT)
            nc.scalar.mul(u1[:], st2[:, 512:1024], INV_CNT)
            nc.scalar.square(u2[:], mean2[:])
            nc.vector.tensor_tensor(u1[:], u1[:], u2[:], op=ALU.subtract)
            nc.scalar.activation(u1[:], u1[:], AF.Sqrt, bias=eps_t[0:1, :])
            nc.vector.reciprocal(u1[:], u1[:])
            nc.vector.tensor_tensor(s2v[:], g2_s[:], u1[:], op=ALU.mult)
            nc.vector.tensor_tensor(u2[:], mean2[:], s2v[:], op=ALU.mult)
            nc.vector.tensor_tensor(t2v[:], b2_s[:], u2[:], op=ALU.subtract)
            s2bc = cp_.tile([128, 512], F32, tag="s2bc", name="s2bc")
            t2bc = cp_.tile([128, 512], F32, tag="t2bc", name="t2bc")
            nc.gpsimd.partition_broadcast(s2bc[:], s2v[:])
            nc.gpsimd.partition_broadcast(t2bc[:], t2v[:])

            # ---- BN2 apply + store
            for img in range(IMGS):
                for t in range(8):
                    yr = scr.tile([128, 512], F32R, tag="s", name="yr")
                    nc.sync.dma_start(yr[:], y_d[img, t])
                    w1 = scr.tile([128, 512], F32, tag="s", name="w1")
                    nc.vector.tensor_tensor(w1[:], yr[:], s2bc[:], op=ALU.mult)
                    nc.vector.tensor_tensor(w1[:], w1[:], t2bc[:], op=ALU.add)
                    nc.vector.tensor_scalar_max(w1[:], w1[:], 0.0)
                    nc.sync.dma_start(out_d[img, 128 * t:128 * t + 128, :],
                                      w1[:])

    nc.compile()
    return nc


_STATE = {}


def _get_nc():
    if "nc" not in _STATE:
        _STATE["nc"] = build_nc()
    return _STATE["nc"]


def make_in_maps(x, prevx, w_prev_qkv, w_qkv, fuse_w, fuse_b, bn1_g, bn1_b,
                 out_w, out_b, bn2_g, bn2_b, gamma, beta):
    f = np.float32
    wq = np.ascontiguousarray(np.asarray(w_qkv, f).T)
    wp = np.ascontiguousarray(np.asarray(w_prev_qkv, f).T)
    fw = np.ascontiguousarray(np.asarray(fuse_w, f))
    ow = np.ascontiguousarray(np.asarray(out_w, f).reshape(9, C, C))
    g = float(np.asarray(gamma, f).reshape(-1)[0])
    g1 = np.ascontiguousarray((g * np.asarray(bn1_g, f)).reshape(4, 128).T)
    b1 = np.ascontiguousarray((g * np.asarray(bn1_b, f)).reshape(4, 128).T)
    g2 = np.ascontiguousarray(np.asarray(bn2_g, f).reshape(1, C))
    b2 = np.ascontiguousarray(np.asarray(bn2_b, f).reshape(1, C))
    bt = float(np.asarray(beta, f).reshape(-1)[0])
    pars = np.array([[bt, 1.0 - bt]], f)
    xf = np.asarray(x, f).reshape(16, N, C)
    pxf = np.asarray(prevx, f).reshape(16, MP, PC)
    maps = []
    for c in range(N_CORES):
        maps.append({
            "x": np.ascontiguousarray(xf[2 * c:2 * c + 2]),
            "px": np.ascontiguousarray(pxf[2 * c:2 * c + 2]),
            "wq": wq, "wp": wp, "fw": fw, "ow": ow,
            "g1": g1, "b1": b1, "g2": g2, "b2": b2, "pars": pars,
        })
    return maps


def kernel(**inputs):
    nc = _get_nc()
    maps = make_in_maps(**inputs)
    res = run_bass_kernel_spmd(nc, maps, list(range(N_CORES)))
    out = np.concatenate([res.results[c]["out"] for c in range(N_CORES)],
                         axis=0)
    return out.reshape(16, 32, 32, C).astype(np.float32)



# revision 12
# speedup vs baseline: 2.6385x; 2.6385x over previous
"""Self-contained Trainium2 Bass kernel for nn_CrossStageAttention.

Data-parallel over batch: 16 images -> 8 NeuronCores x 2 images each.
Training-mode BatchNorm statistics are made global via two tiny AllReduces.

v1 rewrite vs baseline:
  * bf16 data path everywhere (inputs/weights converted on host); matmuls
    run bf16 at 1 cyc/row, DVE elementwise at 2x, all spills eliminated
    (xT / fusx / y stay resident in SBUF -> no DRAM round-trips).
  * softmax row-sums folded into the o-matmuls via a 257-wide augmented-V
    (ones column), killing 384 tiny PE matmuls.
  * avg-pool folded into pre-scaled w_prev (0.25x on host); max path
    compensated via 4x exp-scale and 4x(1-beta) output weight.
  * pooling / eviction work spread across DVE + Act + GpSimd engines.
  * px pipeline runs before self-attention so DVE pooling hides under
    attention matmuls; conv weights prefetched at kernel start.
  * single padded conv input buffer (34x34) with strided matmul lhsT
    access patterns instead of 3 shifted copies.

The torch "(attn@v).transpose(1,2).reshape" scramble is absorbed into the
fuse access patterns (o natural orientation): catT[i, pos=2u+v] = o[512v+i, u].
"""
import numpy as np
import ml_dtypes
from contextlib import ExitStack

import concourse.bass as bass
import concourse.tile as tile
import concourse.bacc as bacc
from concourse import mybir, masks
from concourse.bass_utils import run_bass_kernel_spmd

N_CORES = 8
IMGS = 2
C = 512
N = 1024          # query positions per image (32x32)
PC = 256
MP = 4096         # prev positions per image (64x64)
F32 = mybir.dt.float32
BF = mybir.dt.bfloat16
SCALE = 32 ** -0.5
B0_SELF = 128.0   # constant softmax-stabilization bias for self-attention
EPS = 1e-5
INV_CNT = 1.0 / (16 * 1024)
AF = mybir.ActivationFunctionType
ALU = mybir.AluOpType
X_AXIS = mybir.AxisListType.X


def build_nc():
    nc = bacc.Bacc("TRN2", target_bir_lowering=False, debug=False,
                   num_devices=N_CORES)
    x_d = nc.dram_tensor("x", [IMGS, N, C], BF, kind="ExternalInput").ap()
    px_d = nc.dram_tensor("px", [IMGS, MP, PC], BF, kind="ExternalInput").ap()
    wq_d = nc.dram_tensor("wq", [C, C], BF, kind="ExternalInput").ap()
    wp_d = nc.dram_tensor("wp", [PC, C], BF, kind="ExternalInput").ap()
    fw_d = nc.dram_tensor("fw", [2 * C, C], BF, kind="ExternalInput").ap()
    ow_d = nc.dram_tensor("ow", [9, C, C], BF, kind="ExternalInput").ap()
    g1_d = nc.dram_tensor("g1", [128, 4], F32, kind="ExternalInput").ap()
    b1_d = nc.dram_tensor("b1", [128, 4], F32, kind="ExternalInput").ap()
    g2_d = nc.dram_tensor("g2", [1, C], F32, kind="ExternalInput").ap()
    b2_d = nc.dram_tensor("b2", [1, C], F32, kind="ExternalInput").ap()
    pars_d = nc.dram_tensor("pars", [1, 2], F32, kind="ExternalInput").ap()
    out_d = nc.dram_tensor("out", [IMGS, N, C], F32, kind="ExternalOutput").ap()

    with tile.TileContext(nc) as tc, ExitStack() as ctx:
        const = ctx.enter_context(tc.tile_pool(name="const", bufs=1))
        keep = ctx.enter_context(tc.tile_pool(name="keep", bufs=1))
        scr = ctx.enter_context(tc.tile_pool(name="scr", bufs=5))
        ld = ctx.enter_context(tc.tile_pool(name="ld", bufs=3))
        sm = ctx.enter_context(tc.tile_pool(name="sm", bufs=10))
        ps = ctx.enter_context(tc.tile_pool(name="ps", bufs=4, space="PSUM"))
        psb = ctx.enter_context(tc.tile_pool(name="psb", bufs=2, space="PSUM"))
        dram = ctx.enter_context(tc.tile_pool(name="dram", bufs=1, space="DRAM"))

        # ------------- DRAM scratch (BN stats exchange only) -------------
        bn1_in = dram.tile([128, 8], F32, tag="bn1i")
        bn1_out = dram.tile([128, 8], F32, tag="bn1o")
        bn2_in = dram.tile([1, 1024], F32, tag="bn2i")
        bn2_out = dram.tile([1, 1024], F32, tag="bn2o")

        # ------------- constants / params -------------
        identF = const.tile([128, 128], F32, tag="identF")
        masks.make_identity(nc, identF[:])
        identB = const.tile([128, 128], BF, tag="identB")
        nc.vector.tensor_copy(identB[:], identF[:])
        onesF = const.tile([128, 1], F32, tag="onesF")
        nc.gpsimd.memset(onesF[:], 1.0)
        ones_bf = const.tile([128, 1], BF, tag="onesbf")
        nc.vector.tensor_copy(ones_bf[:], onesF[:])
        b0s = const.tile([128, 1], F32, tag="b0s")
        nc.gpsimd.memset(b0s[:], -B0_SELF)
        eps_t = const.tile([128, 1], F32, tag="eps")
        nc.gpsimd.memset(eps_t[:], EPS)
        g1_s = const.tile([128, 4], F32, tag="g1")
        b1_s = const.tile([128, 4], F32, tag="b1")
        pars_s = const.tile([1, 2], F32, tag="pars")
        pars_bc = const.tile([128, 2], F32, tag="parsbc")
        s1acc = const.tile([128, 4, 4], F32, tag="s1acc")
        ss1acc = const.tile([128, 4, 4], F32, tag="ss1acc")
        s1v = const.tile([128, 4], F32, tag="s1v")
        t1v = const.tile([128, 4], F32, tag="t1v")

        # small params via gpsimd SWDGE; big weights too (keeps the SP
        # queue free for x/px streaming and Act queue free for evictions)
        nc.gpsimd.dma_start(g1_s[:], g1_d)
        nc.gpsimd.dma_start(b1_s[:], b1_d)
        nc.gpsimd.dma_start(pars_s[:], pars_d)
        nc.gpsimd.partition_broadcast(pars_bc[:], pars_s[:])

        wq_s = const.tile([128, 4, C], BF, tag="wq")
        wp_s = const.tile([128, 2, C], BF, tag="wp")
        fw_s = const.tile([128, 8, C], BF, tag="fw")
        ow_s = const.tile([128, 9, 4, C], BF, tag="ow")
        nc.gpsimd.dma_start(wq_s[:], wq_d.rearrange("(ic p) c -> p ic c", p=128))
        nc.gpsimd.dma_start(wp_s[:], wp_d.rearrange("(ic p) c -> p ic c", p=128))
        nc.gpsimd.dma_start(fw_s[:], fw_d.rearrange("(ic p) o -> p ic o", p=128))
        nc.gpsimd.dma_start(ow_s[:],
                            ow_d.rearrange("t (ic p) o -> p t ic o", p=128))

        # persistent per-image tensors (live into the conv phase)
        xT_t = [keep.tile([128, 4, N], BF, tag=f"xT{i}", name=f"xT{i}")
                for i in range(IMGS)]
        fsb_t = [keep.tile([128, 4, N], BF, tag=f"fsb{i}", name=f"fsb{i}")
                 for i in range(IMGS)]
        y_s = keep.tile([128, 16, C], BF, tag="ys")

        # =================== attention scope ===================
        with tc.tile_pool(name="attn", bufs=1) as ap_:
            for img in range(IMGS):
                xT = xT_t[img]
                qT = ap_.tile([128, 4, N], BF, tag="qT", name="qT")
                avgT = ap_.tile([128, 4, N], BF, tag="avgT", name="avgT")
                maxT = ap_.tile([128, 4, N], BF, tag="maxT", name="maxT")
                xnow_t = ap_.tile([128, 8, C], BF, tag="xnow", name="xnow")
                xprev_t = ap_.tile([128, 8, C], BF, tag="xprev", name="xprev")

                # ---- x load + PE transpose -> xT (bf16, resident)
                for nt in range(8):
                    xl = ld.tile([128, C], BF, tag="xl", name="xl")
                    nc.sync.dma_start(xl[:], x_d[img, 128 * nt:128 * nt + 128, :])
                    pt = psb.tile([128, 512], BF, tag="pt", name="ptx")
                    for ci in range(4):
                        nc.tensor.transpose(pt[:, 128 * ci:128 * ci + 128],
                                            xl[:, 128 * ci:128 * ci + 128],
                                            identB[:])
                    dst = xT[:, :, 128 * nt:128 * nt + 128]
                    src = pt[:].rearrange("p (ci n) -> p ci n", ci=4)
                    if nt % 2 == 0:
                        nc.vector.tensor_copy(dst, src)
                    else:
                        nc.scalar.copy(dst, src)

                # ---- qT projection (wq lhsT x xT)
                for ci in range(4):
                    for nh in range(2):
                        qp = ps.tile([128, 512], F32, tag="b", name="qp")
                        for ic in range(4):
                            nc.tensor.matmul(
                                qp[:], wq_s[:, ic, 128 * ci:128 * ci + 128],
                                xT[:, ic, 512 * nh:512 * nh + 512],
                                start=(ic == 0), stop=(ic == 3))
                        nc.scalar.copy(qT[:, ci, 512 * nh:512 * nh + 512],
                                       qp[:])

                # ---- px pipeline: transpose, project, pool (before
                #      self-attn so DVE pooling hides under attention mms)
                for ch in range(8):
                    pxc = ap_.tile([128, 2, 512], BF, tag="pxc", bufs=2,
                                   name="pxc")
                    for kk in range(4):
                        pl = ld.tile([128, PC], BF, tag="pl", bufs=4,
                                     name="pl")
                        nc.sync.dma_start(
                            pl[:],
                            px_d[img, 512 * ch + 128 * kk:
                                 512 * ch + 128 * kk + 128, :])
                        ptp = psb.tile([128, 512], BF, tag="pt", name="ptp")
                        for pc in range(2):
                            nc.tensor.transpose(
                                ptp[:, 128 * pc:128 * pc + 128],
                                pl[:, 128 * pc:128 * pc + 128], identB[:])
                        dst = pxc[:, :, 128 * kk:128 * kk + 128]
                        src = ptp[:, 0:256].rearrange("p (pc n) -> p pc n",
                                                      pc=2)
                        nc.vector.tensor_copy(dst, src)
                    for ci in range(4):
                        pq = ps.tile([128, 512], F32, tag="b", name="pq")
                        for pc in range(2):
                            nc.tensor.matmul(
                                pq[:], wp_s[:, pc, 128 * ci:128 * ci + 128],
                                pxc[:, pc, :],
                                start=(pc == 0), stop=(pc == 1))
                        # fast Act eviction frees the PSUM bank; pooling
                        # runs on the bf16 SBUF copy (DVE 2x + GpSimd)
                        pqs = scr.tile([128, 512], BF, tag="pqs", bufs=4,
                                       name="pqs")
                        nc.scalar.copy(pqs[:], pq[:])
                        v = pqs[:].rearrange("p (i a j b) -> p i a j b",
                                             i=4, a=2, j=32, b=2)
                        mx1 = scr.tile([128, 256], BF, tag="p256", bufs=4,
                                       name="mx1")
                        mv = mx1[:].rearrange("p (i a j) -> p i a j",
                                              i=4, a=2)
                        nc.vector.tensor_tensor(mv, v[:, :, :, :, 0],
                                                v[:, :, :, :, 1], op=ALU.max)
                        nc.vector.tensor_tensor(
                            maxT[:, ci, 128 * ch:128 * ch + 128]
                            .rearrange("p (i j) -> p i j", j=32),
                            mv[:, :, 0, :], mv[:, :, 1, :], op=ALU.max)
                        # avg: wp pre-scaled 0.25 so plain sums suffice
                        av1 = scr.tile([128, 256], BF, tag="p256", bufs=4,
                                       name="av1")
                        avv = av1[:].rearrange("p (i a j) -> p i a j",
                                               i=4, a=2)
                        nc.vector.tensor_tensor(avv, v[:, :, :, :, 0],
                                                v[:, :, :, :, 1], op=ALU.add)
                        nc.vector.tensor_tensor(
                            avgT[:, ci, 128 * ch:128 * ch + 128]
                            .rearrange("p (i j) -> p i j", j=32),
                            avv[:, :, 0, :], avv[:, :, 1, :], op=ALU.add)

                # ---- augmented V in natural orientation (ones col at
                #      256 and 513 -> row-sums fall out of the o-matmuls)
                def vaug_ones(va):
                    nc.gpsimd.memset(va[:, :, 256:257], 1.0)
                    nc.gpsimd.memset(va[:, :, 513:514], 1.0)

                def evict_vaug(va, mi, src, eng):
                    if eng == "act":
                        nc.scalar.copy(va[:, mi, 0:256], src[:, 0:256])
                        nc.scalar.copy(va[:, mi, 257:513], src[:, 256:512])
                    else:
                        nc.vector.tensor_copy(va[:, mi, 0:256], src[:, 0:256])
                        nc.vector.tensor_copy(va[:, mi, 257:513],
                                              src[:, 256:512])

                # self-attention V = qkv natural, via matmul from xT
                vaug = ap_.tile([128, 8, 514], BF, tag="vaug", name="vaug_s")
                vaug_ones(vaug)
                for nt in range(8):
                    vp = ps.tile([128, 512], F32, tag="b", name="vp")
                    for ic in range(4):
                        nc.tensor.matmul(
                            vp[:], xT[:, ic, 128 * nt:128 * nt + 128],
                            wq_s[:, ic, :],
                            start=(ic == 0), stop=(ic == 3))
                    evict_vaug(vaug, nt, vp[:], "act" if nt % 2 else "vec")

                def do_attn(kind, kvT, va):
                    bias = b0s[:] if kind == "self" else 0.0
                    scale = SCALE * (4.0 if kind == "max" else 1.0)
                    for nh in range(2):
                        eas = []
                        for mi in range(8):
                            lg = ps.tile([128, 512], F32, tag="b", name="lg")
                            for ci in range(4):
                                nc.tensor.matmul(
                                    lg[:],
                                    kvT[:, ci, 128 * mi:128 * mi + 128],
                                    qT[:, ci, 512 * nh:512 * nh + 512],
                                    start=(ci == 0), stop=(ci == 3))
                            ea = scr.tile([128, 512], BF, tag="ea", bufs=9,
                                          name="ea")
                            nc.scalar.activation(ea[:], lg[:], AF.Exp,
                                                 bias=bias, scale=scale)
                            eas.append(ea)
                        for np2 in range(2):
                            for k in range(2):
                                oa = ps.tile([128, 512], F32, tag="b",
                                             name="oa")
                                ob = ps.tile([128, 512], F32, tag="b",
                                             name="ob")
                                for mi in range(8):
                                    lhsT = eas[mi][:, 128 * (2 * np2 + k):
                                                   128 * (2 * np2 + k) + 128]
                                    nc.tensor.matmul(oa[:, 0:257], lhsT,
                                                     va[:, mi, 0:257],
                                                     start=(mi == 0),
                                                     stop=(mi == 7))
                                    nc.tensor.matmul(ob[:, 0:257], lhsT,
                                                     va[:, mi, 257:514],
                                                     start=(mi == 0),
                                                     stop=(mi == 7))
                                nck = 4 * nh + 2 * np2 + k
                                rec = sm.tile([128, 1], F32, name="rec")
                                nc.vector.reciprocal(rec[:], oa[:, 256:257])
                                if kind == "self":
                                    w = rec
                                elif kind == "avg":
                                    w = sm.tile([128, 1], F32, name="bw")
                                    nc.vector.tensor_tensor(
                                        w[:], rec[:], pars_bc[:, 0:1],
                                        op=ALU.mult)
                                else:
                                    w = sm.tile([128, 1], F32, name="bw")
                                    nc.vector.tensor_tensor(
                                        w[:], rec[:], pars_bc[:, 1:2],
                                        op=ALU.mult)
                                if kind == "max":
                                    t_ = scr.tile([128, 512], BF, tag="s",
                                                  name="mx")
                                    nc.scalar.mul(t_[:, 0:256],
                                                  oa[:, 0:256], w[:])
                                    nc.vector.tensor_scalar_mul(
                                        t_[:, 256:512], ob[:, 0:256], w[:])
                                    nc.vector.tensor_tensor(
                                        xprev_t[:, nck, :],
                                        xprev_t[:, nck, :], t_[:],
                                        op=ALU.add)
                                else:
                                    dstt = (xnow_t if kind == "self"
                                            else xprev_t)
                                    nc.scalar.mul(dstt[:, nck, 0:256],
                                                  oa[:, 0:256], w[:])
                                    nc.vector.tensor_scalar_mul(
                                        dstt[:, nck, 256:512],
                                        ob[:, 0:256], w[:])

                do_attn("self", qT, vaug)

                # avg attention: rebuild vaug by transposing avgT
                vaug = ap_.tile([128, 8, 514], BF, tag="vaug", name="vaug_a")
                vaug_ones(vaug)
                for mi in range(8):
                    ptn = psb.tile([128, 512], BF, tag="pt", name="ptn")
                    for ci in range(4):
                        nc.tensor.transpose(
                            ptn[:, 128 * ci:128 * ci + 128],
                            avgT[:, ci, 128 * mi:128 * mi + 128], identB[:])
                    evict_vaug(vaug, mi, ptn[:], "act" if mi % 2 else "vec")
                do_attn("avg", avgT, vaug)

                # max attention
                vaug = ap_.tile([128, 8, 514], BF, tag="vaug", name="vaug_m")
                vaug_ones(vaug)
                for mi in range(8):
                    ptn = psb.tile([128, 512], BF, tag="pt", name="ptm")
                    for ci in range(4):
                        nc.tensor.transpose(
                            ptn[:, 128 * ci:128 * ci + 128],
                            maxT[:, ci, 128 * mi:128 * mi + 128], identB[:])
                    evict_vaug(vaug, mi, ptn[:], "act" if mi % 2 else "vec")
                do_attn("max", maxT, vaug)

                # ---- fuse matmul + BN1 partial stats; fusx resident,
                #      stored position-interleaved: fsb[:, oi, 2u+v]
                fsb = fsb_t[img]
                for oi in range(4):
                    for v in range(2):
                        fp = ps.tile([128, 512], F32, tag="b", name="fp")
                        for ii in range(8):
                            rhs = (xnow_t[:, 4 * v + ii, :] if ii < 4
                                   else xprev_t[:, 4 * v + (ii - 4), :])
                            nc.tensor.matmul(
                                fp[:], fw_s[:, ii, 128 * oi:128 * oi + 128],
                                rhs, start=(ii == 0), stop=(ii == 7))
                        slot = 2 * img + v
                        dst = (fsb[:, oi, :]
                               .rearrange("p (u two) -> p u two", two=2)
                               [:, :, v])
                        nc.scalar.activation(
                            dst, fp[:], AF.Copy,
                            accum_out=s1acc[:, oi, slot:slot + 1])
                        sqt = scr.tile([128, 512], BF, tag="s", name="sqt")
                        nc.scalar.activation(
                            sqt[:], fp[:], AF.Square,
                            accum_out=ss1acc[:, oi, slot:slot + 1])

        # =================== BN1 global stats ===================
        sum1 = sm.tile([128, 4], F32, name="sum1")
        ssq1 = sm.tile([128, 4], F32, name="ssq1")
        nc.vector.tensor_reduce(sum1[:], s1acc[:], axis=X_AXIS, op=ALU.add)
        nc.vector.tensor_reduce(ssq1[:], ss1acc[:], axis=X_AXIS, op=ALU.add)
        nc.gpsimd.dma_start(bn1_in[:, 0:4], sum1[:])
        nc.gpsimd.dma_start(bn1_in[:, 4:8], ssq1[:])
        nc.gpsimd.collective_compute(
            "AllReduce", ALU.add, replica_groups=[list(range(N_CORES))],
            ins=[bn1_in.opt()], outs=[bn1_out.opt()])
        allst = sm.tile([128, 8], F32, name="allst")
        nc.sync.dma_start(allst[:], bn1_out[:])
        mean1 = sm.tile([128, 4], F32, name="mean1")
        tA = sm.tile([128, 4], F32, name="tA")
        tB = sm.tile([128, 4], F32, name="tB")
        nc.scalar.mul(mean1[:], allst[:, 0:4], INV_CNT)
        nc.scalar.mul(tA[:], allst[:, 4:8], INV_CNT)
        nc.scalar.square(tB[:], mean1[:])
        nc.vector.tensor_tensor(tA[:], tA[:], tB[:], op=ALU.subtract)
        nc.scalar.activation(tA[:], tA[:], AF.Sqrt, bias=eps_t[:])
        nc.vector.reciprocal(tA[:], tA[:])
        nc.vector.tensor_tensor(s1v[:], g1_s[:], tA[:], op=ALU.mult)
        nc.vector.tensor_tensor(tB[:], mean1[:], s1v[:], op=ALU.mult)
        nc.vector.tensor_tensor(t1v[:], b1_s[:], tB[:], op=ALU.subtract)

        # =================== conv scope ===================
        with tc.tile_pool(name="conv", bufs=1) as cp_, \
                tc.tile_pool(name="rows", bufs=1) as rows:
            # 3 column-shifted, vertically padded buffers per image
            # (matmul lhsT needs a single free dim):
            #   x2s[:, d, ci, r*32 + w] = x2[r-1, w+d-1]  (0 outside)
            # center (d=1) = residual add; d=0/2 = shifted copies + strip
            x2ss = []
            for img in range(IMGS):
                x2s = cp_.tile([128, 3, 4, 1088], BF, tag=f"x2s{img}",
                               name=f"x2s{img}")
                x2ss.append(x2s)
                for ci in range(4):
                    rt = scr.tile([128, N], BF, tag="rt", bufs=2, name="rt")
                    nc.scalar.activation(rt[:], fsb_t[img][:, ci, :],
                                         AF.Relu, bias=t1v[:, ci:ci + 1],
                                         scale=s1v[:, ci:ci + 1])
                    ctr = x2s[:, 1, ci, :]
                    nc.gpsimd.memset(ctr[0:128, 0:32], 0.0)
                    nc.gpsimd.memset(ctr[0:128, 1056:1088], 0.0)
                    nc.vector.tensor_tensor(ctr[0:128, 32:1056], rt[:],
                                            xT_t[img][:, ci, :], op=ALU.add)
                    nc.vector.tensor_copy(x2s[:, 0, ci, 1:1088],
                                          ctr[0:128, 0:1087])
                    nc.gpsimd.memset(
                        x2s[:, 0, ci, :].rearrange("p (r w) -> p r w", w=32)
                        [:, :, 0], 0.0)
                    nc.scalar.copy(x2s[:, 2, ci, 0:1087],
                                   ctr[0:128, 1:1088])
                    nc.gpsimd.memset(
                        x2s[:, 2, ci, :].rearrange("p (r w) -> p r w", w=32)
                        [:, :, 31], 0.0)

            # ---- conv 3x3 + BN2 stats (y resident in SBUF)
            sy_ps = ps.tile([128, 512], F32, tag="st", bufs=1, name="syp")
            sy2_ps = ps.tile([128, 512], F32, tag="st2", bufs=1, name="sy2p")
            first = True
            for img in range(IMGS):
                for t in range(8):
                    yp = ps.tile([128, 512], F32, tag="b", name="yp")
                    k = 0
                    for tap in range(9):
                        dh, dw = tap // 3, tap % 3
                        for ci in range(4):
                            lhsT = x2ss[img][:, dw, ci,
                                             32 * (4 * t + dh):
                                             32 * (4 * t + dh) + 128]
                            nc.tensor.matmul(yp[:], lhsT,
                                             ow_s[:, tap, ci, :],
                                             start=(k == 0), stop=(k == 35))
                            k += 1
                    slot = 8 * img + t
                    if t % 2 == 0:
                        nc.vector.tensor_copy(y_s[:, slot, :], yp[:])
                    else:
                        nc.scalar.copy(y_s[:, slot, :], yp[:])
                    ysq = scr.tile([128, 512], BF, tag="s", name="ysq")
                    nc.scalar.square(ysq[:], yp[:])
                    last = (img == IMGS - 1 and t == 7)
                    nc.tensor.matmul(sy_ps[0:1, :], ones_bf[:],
                                     y_s[:, slot, :],
                                     start=first, stop=last)
                    nc.tensor.matmul(sy2_ps[0:1, :], ones_bf[:], ysq[:],
                                     start=first, stop=last)
                    first = False

            # ---- BN2 global stats
            syr = rows.tile([1, 1024], F32, tag="syr", bufs=1, name="syr")
            nc.vector.tensor_copy(syr[:, 0:512], sy_ps[0:1, :])
            nc.vector.tensor_copy(syr[:, 512:1024], sy2_ps[0:1, :])
            nc.gpsimd.dma_start(bn2_in[:, :], syr[:])
            nc.gpsimd.collective_compute(
                "AllReduce", ALU.add, replica_groups=[list(range(N_CORES))],
                ins=[bn2_in.opt()], outs=[bn2_out.opt()])
            # in-place row math: st2[0:512] ends as mean2*s2v, st2[512:]
            # as 1/std; g2s ends as s2v, b2s as t2v
            st2 = rows.tile([1, 1024], F32, tag="st2", bufs=1, name="st2")
            nc.sync.dma_start(st2[:], bn2_out[:])
            g2_s = rows.tile([1, C], F32, tag="g2s", bufs=1, name="g2s")
            b2_s = rows.tile([1, C], F32, tag="b2s", bufs=1, name="b2s")
            nc.gpsimd.dma_start(g2_s[:], g2_d)
            nc.gpsimd.dma_start(b2_s[:], b2_d)
            u2 = rows.tile([1, 512], F32, tag="u2", bufs=1, name="u2")
            nc.scalar.mul(st2[:, 0:512], st2[:, 0:512], INV_CNT)
            nc.scalar.mul(st2[:, 512:1024], st2[:, 512:1024], INV_CNT)
            nc.scalar.square(u2[:], st2[:, 0:512])
            nc.vector.tensor_tensor(st2[:, 512:1024], st2[:, 512:1024],
                                    u2[:], op=ALU.subtract)
            nc.scalar.activation(st2[:, 512:1024], st2[:, 512:1024],
                                 AF.Sqrt, bias=eps_t[0:1, :])
            nc.vector.reciprocal(st2[:, 512:1024], st2[:, 512:1024])
            nc.vector.tensor_tensor(g2_s[:], g2_s[:], st2[:, 512:1024],
                                    op=ALU.mult)
            nc.vector.tensor_tensor(st2[:, 0:512], st2[:, 0:512], g2_s[:],
                                    op=ALU.mult)
            nc.vector.tensor_tensor(b2_s[:], b2_s[:], st2[:, 0:512],
                                    op=ALU.subtract)
            s2b = rows.tile([1, 512], BF, tag="s2b", bufs=1, name="s2b")
            t2b = rows.tile([1, 512], BF, tag="t2b", bufs=1, name="t2b")
            nc.vector.tensor_copy(s2b[:], g2_s[:])
            nc.vector.tensor_copy(t2b[:], b2_s[:])
            s2bc = cp_.tile([128, 512], BF, tag="s2bc", name="s2bc")
            t2bc = cp_.tile([128, 512], BF, tag="t2bc", name="t2bc")
            nc.gpsimd.partition_broadcast(s2bc[:], s2b[:])
            nc.gpsimd.partition_broadcast(t2bc[:], t2b[:])

            # ---- BN2 apply + store (y already resident)
            for img in range(IMGS):
                for t in range(8):
                    slot = 8 * img + t
                    m1 = scr.tile([128, 512], BF, tag="s", name="m1")
                    nc.vector.tensor_tensor(m1[:], y_s[:, slot, :],
                                            s2bc[:], op=ALU.mult)
                    m2 = scr.tile([128, 512], BF, tag="s", name="m2")
                    nc.vector.tensor_tensor(m2[:], m1[:], t2bc[:],
                                            op=ALU.add)
                    w1 = scr.tile([128, 512], F32, tag="w1", bufs=2,
                                  name="w1")
                    if t % 2 == 0:
                        nc.scalar.activation(w1[:], m2[:], AF.Relu)
                    else:
                        nc.vector.tensor_scalar_max(w1[:], m2[:], 0.0)
                    nc.sync.dma_start(out_d[img, 128 * t:128 * t + 128, :],
                                      w1[:])

    nc.compile()
    return nc


_STATE = {}


def _get_nc():
    if "nc" not in _STATE:
        _STATE["nc"] = build_nc()
    return _STATE["nc"]


def make_in_maps(x, prevx, w_prev_qkv, w_qkv, fuse_w, fuse_b, bn1_g, bn1_b,
                 out_w, out_b, bn2_g, bn2_b, gamma, beta):
    f = np.float32
    bf = ml_dtypes.bfloat16
    wq = np.ascontiguousarray(np.asarray(w_qkv, f).T.astype(bf))
    # 0.25x: folds the avg-pool normalization into the projection; the
    # max path is compensated by 4x exp-scale and 4x(1-beta) weight.
    wp = np.ascontiguousarray((0.25 * np.asarray(w_prev_qkv, f).T).astype(bf))
    fw = np.ascontiguousarray(np.asarray(fuse_w, f).astype(bf))
    ow = np.ascontiguousarray(np.asarray(out_w, f).reshape(9, C, C).astype(bf))
    g = float(np.asarray(gamma, f).reshape(-1)[0])
    g1 = np.ascontiguousarray((g * np.asarray(bn1_g, f)).reshape(4, 128).T)
    b1 = np.ascontiguousarray((g * np.asarray(bn1_b, f)).reshape(4, 128).T)
    g2 = np.ascontiguousarray(np.asarray(bn2_g, f).reshape(1, C))
    b2 = np.ascontiguousarray(np.asarray(bn2_b, f).reshape(1, C))
    bt = float(np.asarray(beta, f).reshape(-1)[0])
    pars = np.array([[bt, 4.0 * (1.0 - bt)]], f)
    xf = np.asarray(x, f).reshape(16, N, C).astype(bf)
    pxf = np.asarray(prevx, f).reshape(16, MP, PC).astype(bf)
    maps = []
    for c in range(N_CORES):
        maps.append({
            "x": np.ascontiguousarray(xf[2 * c:2 * c + 2]),
            "px": np.ascontiguousarray(pxf[2 * c:2 * c + 2]),
            "wq": wq, "wp": wp, "fw": fw, "ow": ow,
            "g1": g1, "b1": b1, "g2": g2, "b2": b2, "pars": pars,
        })
    return maps


def kernel(**inputs):
    nc = _get_nc()
    maps = make_in_maps(**inputs)
    res = run_bass_kernel_spmd(nc, maps, list(range(N_CORES)))
    out = np.concatenate([res.results[c]["out"] for c in range(N_CORES)],
                         axis=0)
    return out.reshape(16, 32, 32, C).astype(np.float32)
